# revision 11
# baseline (speedup 1.0000x reference)
"""Differential attention kernel for 8 Trainium2 NeuronCores.

Sharding: tensor-parallel over heads (2 q-heads + their shared kv-head per
core) for QKV projection + attention; the output projection is sequence-
sharded after an AllToAll that redistributes y from head-sharded to
T-sharded layout. Host only slices weights / concatenates output blocks.

Compute layout is "transposed land": activations live as [feature, T] in
SBUF so every matmul contracts over the partition axis. Scores are built
as sT[k, q] (k-stationary, K=32 RoPE halves row-packed 2-way), softmax is
max-free (scores ~ N(0,1)), denominators come from a ones-column in the
AV stationary, and the a1 - lam*a2 combination uses the scale-invariance
of RMSNorm to avoid any division:  rmsnorm(Z2*u1 - lam*Z1*u2) ==
rmsnorm(u1/Z1 - lam*u2/Z2).
"""

import os
import sys
import math

import numpy as np
import ml_dtypes

sys.path.insert(0, "/opt/trn_rl_repo")

import concourse.bass as bass  # noqa: E402
import concourse.bacc as bacc  # noqa: E402
import concourse.mybir as mybir  # noqa: E402
import concourse.tile as tile  # noqa: E402
from concourse import bass_utils  # noqa: E402

F32 = mybir.dt.float32
BF16 = mybir.dt.bfloat16
AF = mybir.ActivationFunctionType
OP = mybir.AluOpType
NPBF16 = ml_dtypes.bfloat16

N_CORES = 8
T = 2048
C = 1024
N_HEADS = 16
N_KV_HEADS = 4
HD = 64
HALF = 32
EPS = 1e-6
SCALE = HALF ** -0.5

DEBUG_TAPS = False    # extra dram outputs for debugging

ST = 512              # query supertile
NST = T // ST         # 4 supertiles
QT = 128              # query subtile (mask granularity)
NKT = T // 128        # 16 key tiles
HPC = 2               # q heads per core


def _build_nc():
    nc = bacc.Bacc("TRN2", target_bir_lowering=False, debug=False,
                   num_devices=N_CORES)

    x_d = nc.dram_tensor("x", [T, C], F32, kind="ExternalInput")
    wq_d = nc.dram_tensor("wq", [128, C], F32, kind="ExternalInput")
    wkv_d = nc.dram_tensor("wkv", [128, C], F32, kind="ExternalInput")
    wout_d = nc.dram_tensor("wout", [C, C], F32, kind="ExternalInput")
    lam_d = nc.dram_tensor("lam2", [1, 2], F32, kind="ExternalInput")
    rmsw_d = nc.dram_tensor("rmsw", [64, 1], F32, kind="ExternalInput")
    cos_d = nc.dram_tensor("cos16", [16, T], BF16, kind="ExternalInput")
    sin_d = nc.dram_tensor("sin16", [16, T], BF16, kind="ExternalInput")
    tri_d = nc.dram_tensor("tri", [128, 128], BF16, kind="ExternalInput")
    ones2_d = nc.dram_tensor("ones2", [128, 2], BF16, kind="ExternalInput")
    out_d = nc.dram_tensor("out", [256, C], F32, kind="ExternalOutput")
    dbg = {}
    if DEBUG_TAPS:
        for nm, shp, dt in [("d_qTe", [64, T], BF16), ("d_qTo", [64, T], BF16),
                            ("d_re", [64, T], BF16), ("d_ro", [64, T], BF16),
                            ("d_krep", [64, T], BF16), ("d_qrep0", [64, T], BF16),
                            ("d_vps", [128, 1280], BF16), ("d_e", [128, 1024], BF16),
                            ("d_zrow", [1, 1024], F32), ("d_ybuf", [128, T], BF16),
                            ("d_ssq", [1, 2048], F32), ("d_yTloc", [128, 1024], BF16),
                            ("d_xT", [128, 2048], BF16),
                            ("d_lnb", [1, 2048], F32), ("d_rstd2", [1, 2048], F32),
                            ("d_ynorm", [128, 1024], BF16)]:
            dbg[nm] = nc.dram_tensor(nm, shp, dt, kind="ExternalOutput")

    with tile.TileContext(nc) as tc:
        with tc.tile_pool(name="big", bufs=1) as big, \
             tc.tile_pool(name="stage", bufs=3) as stage, \
             tc.tile_pool(name="ework", bufs=3) as ework, \
             tc.tile_pool(name="yw", bufs=2) as yw, \
             tc.tile_pool(name="yw1", bufs=1) as yw1, \
             tc.tile_pool(name="ps", bufs=1, space="PSUM") as ps, \
             tc.tile_pool(name="ps2", bufs=2, space="PSUM") as ps2, \
             tc.tile_pool(name="dram", bufs=2, space="DRAM") as dram:

            # ---------- persistent SBUF tiles ----------
            xT = big.tile([128, 8 * T], BF16, tag="xT")          # 32KB/p
            wqT = big.tile([128, C], BF16, tag="wqT")
            wkvT = big.tile([128, C], BF16, tag="wkvT")
            woutT = big.tile([128, 8 * C], BF16, tag="woutT")    # 16KB/p
            qTe = big.tile([64, T], BF16, tag="qTe")
            qTo = big.tile([64, T], BF16, tag="qTo")
            kTe = big.tile([32, T], BF16, tag="kTe")
            kTo = big.tile([32, T], BF16, tag="kTo")
            vT = big.tile([64, T], BF16, tag="vT")
            cosR = big.tile([64, T], BF16, tag="cosR")
            sinR = big.tile([64, T], BF16, tag="sinR")
            qrep = [big.tile([64, T], BF16, tag=f"qrep{h}", name=f"qrep{h}")
                    for h in range(2)]
            krep = big.tile([64, T], BF16, tag="krep")
            vps = big.tile([128, 16 * 80], BF16, tag="vps")
            tri = big.tile([128, 128], BF16, tag="tri")
            ones2 = big.tile([128, 2], BF16, tag="ones2")
            ybuf = big.tile([128, T], BF16, tag="ybuf")
            ssqbuf = big.tile([1, 2048], F32, tag="ssqbuf")
            rw128 = big.tile([128, 1], F32, tag="rw128")
            lamt = big.tile([1, 2], F32, tag="lamt")
            epsb = big.tile([1, 1], F32, tag="epsb")
            neglam64 = [big.tile([64, 1], F32, tag=f"nl64{h}", name=f"nl64{h}")
                        for h in range(2)]

            sync = nc.sync
            gp = nc.gpsimd
            ve = nc.vector
            se = nc.scalar
            te = nc.tensor

            # ---------- constants ----------
            sync.dma_start(out=tri[:], in_=tri_d.ap())
            sync.dma_start(out=ones2[:], in_=ones2_d.ap())
            for b in range(4):
                sync.dma_start(out=cosR[b * 16:(b + 1) * 16, :], in_=cos_d.ap())
                sync.dma_start(out=sinR[b * 16:(b + 1) * 16, :], in_=sin_d.ap())
            ve.memset(epsb[:], EPS)
            sync.dma_start(out=rw128[0:64, :], in_=rmsw_d.ap())
            sync.dma_start(out=rw128[64:128, :], in_=rmsw_d.ap())

            # lam = sigmoid(lambda_init); store -lam broadcast per head
            lraw = stage.tile([1, 2], F32, tag="lraw")
            sync.dma_start(out=lraw[:], in_=lam_d.ap())
            se.activation(out=lamt[:], in_=lraw[:], func=AF.Exp, scale=-1.0)
            ve.tensor_scalar(out=lamt[:], in0=lamt[:], scalar1=1.0, scalar2=None,
                             op0=OP.add)
            ve.reciprocal(out=lamt[:], in_=lamt[:])
            ve.tensor_scalar(out=lamt[:], in0=lamt[:], scalar1=-1.0, scalar2=None,
                             op0=OP.mult)
            for h in range(2):
                nl1 = stage.tile([1, 1], F32, tag="nl1", name=f"nl1_{h}")
                ve.tensor_copy(out=nl1[:], in_=lamt[0:1, h:h + 1])
                gp.partition_broadcast(neglam64[h][:], nl1[:])

            # ---------- x: cast to bf16 and transpose into xT ----------
            for i in range(16):
                xn = stage.tile([128, C], BF16, tag="xn")
                gp.dma_start(out=xn[:], in_=x_d.ap()[i * 128:(i + 1) * 128, :])
                for c in range(8):
                    sync.dma_start_transpose(
                        out=xT[:, c * T + i * 128: c * T + (i + 1) * 128],
                        in_=xn[:, c * 128:(c + 1) * 128])

            # ---------- weights: permuted cast loads + transposes ----------
            # wq rows reordered to [all-even-pairs | all-odd-pairs]
            wq_perm = wq_d.ap().rearrange("(b i p) c -> p b i c", b=4, i=16, p=2)
            wqn = stage.tile([128, C], BF16, tag="wn")
            gp.dma_start(out=wqn[:], in_=wq_perm)
            for c in range(8):
                sync.dma_start_transpose(out=wqT[:, c * 128:(c + 1) * 128],
                                         in_=wqn[:, c * 128:(c + 1) * 128])
            wkv_perm = (wkv_d.ap()[0:64, :]
                        .rearrange("(b i p) c -> p b i c", b=2, i=16, p=2))
            wkvn = stage.tile([128, C], BF16, tag="wn")
            gp.dma_start(out=wkvn[0:64, :], in_=wkv_perm)
            gp.dma_start(out=wkvn[64:128, :], in_=wkv_d.ap()[64:128, :])
            for c in range(8):
                sync.dma_start_transpose(out=wkvT[:, c * 128:(c + 1) * 128],
                                         in_=wkvn[:, c * 128:(c + 1) * 128])
            for r in range(8):
                won = stage.tile([128, C], BF16, tag="wn")
                gp.dma_start(out=won[:], in_=wout_d.ap()[r * 128:(r + 1) * 128, :])
                for c in range(8):
                    sync.dma_start_transpose(
                        out=woutT[:, c * C + r * 128: c * C + (r + 1) * 128],
                        in_=won[:, c * 128:(c + 1) * 128])
            # fold rms_weight into woutT rows (rows of woutT chunk = C dims)
            for c in range(8):
                ve.tensor_scalar(out=woutT[:, c * C:(c + 1) * C],
                                 in0=woutT[:, c * C:(c + 1) * C],
                                 scalar1=rw128[:], scalar2=None, op0=OP.mult)

            # ---------- QKV projections (bf16, psum f32) ----------
            for n in range(4):
                qp = ps.tile([128, 512], F32, tag="mm512")
                for c in range(8):
                    te.matmul(out=qp[:], lhsT=wqT[:, c * 128:(c + 1) * 128],
                              rhs=xT[:, c * T + n * 512: c * T + (n + 1) * 512],
                              start=(c == 0), stop=(c == 7))
                ve.tensor_copy(out=qTe[:, n * 512:(n + 1) * 512], in_=qp[0:64, :])
                ve.tensor_copy(out=qTo[:, n * 512:(n + 1) * 512], in_=qp[64:128, :])
            for n in range(4):
                kp = ps.tile([128, 512], F32, tag="mm512")
                for c in range(8):
                    te.matmul(out=kp[:], lhsT=wkvT[:, c * 128:(c + 1) * 128],
                              rhs=xT[:, c * T + n * 512: c * T + (n + 1) * 512],
                              start=(c == 0), stop=(c == 7))
                ve.tensor_copy(out=kTe[:, n * 512:(n + 1) * 512], in_=kp[0:32, :])
                ve.tensor_copy(out=kTo[:, n * 512:(n + 1) * 512], in_=kp[32:64, :])
                ve.tensor_copy(out=vT[:, n * 512:(n + 1) * 512], in_=kp[64:128, :])

            if DEBUG_TAPS:
                sync.dma_start(out=dbg["d_qTe"].ap(), in_=qTe[:])
                sync.dma_start(out=dbg["d_qTo"].ap(), in_=qTo[:])
                sync.dma_start(out=dbg["d_xT"].ap(), in_=xT[:, 0:2048])

            # ---------- RoPE (layout: evens/odds in separate tiles) ----------
            qre = big.tile([64, T], BF16, tag="qre")
            qro = big.tile([64, T], BF16, tag="qro")
            kre = big.tile([32, T], BF16, tag="kre")
            kro = big.tile([32, T], BF16, tag="kro")
            for n in range(4):
                s = slice(n * 512, (n + 1) * 512)
                for (e_in, o_in, e_out, o_out, nr) in (
                        (qTe, qTo, qre, qro, 64), (kTe, kTo, kre, kro, 32)):
                    t1 = stage.tile([64, 512], BF16, tag="rt1")
                    t2 = stage.tile([64, 512], BF16, tag="rt2")
                    ve.tensor_tensor(out=t1[0:nr, :], in0=e_in[0:nr, s],
                                     in1=cosR[0:nr, s], op=OP.mult)
                    ve.tensor_tensor(out=t2[0:nr, :], in0=o_in[0:nr, s],
                                     in1=sinR[0:nr, s], op=OP.mult)
                    ve.tensor_tensor(out=e_out[0:nr, s], in0=t1[0:nr, :],
                                     in1=t2[0:nr, :], op=OP.subtract)
                    ve.tensor_tensor(out=t1[0:nr, :], in0=e_in[0:nr, s],
                                     in1=sinR[0:nr, s], op=OP.mult)
                    ve.tensor_tensor(out=t2[0:nr, :], in0=o_in[0:nr, s],
                                     in1=cosR[0:nr, s], op=OP.mult)
                    ve.tensor_tensor(out=o_out[0:nr, s], in0=t1[0:nr, :],
                                     in1=t2[0:nr, :], op=OP.add)

            # score-layout rearrangement (16-row block moves, via DMA)
            for h in range(2):
                for q2 in range(2):
                    b = h * 2 + q2
                    sync.dma_start(out=qrep[h][q2 * 32:q2 * 32 + 16, :],
                                   in_=qre[b * 16:(b + 1) * 16, :])
                    sync.dma_start(out=qrep[h][q2 * 32 + 16:q2 * 32 + 32, :],
                                   in_=qro[b * 16:(b + 1) * 16, :])
            for q2 in range(2):
                sync.dma_start(out=krep[q2 * 32:q2 * 32 + 16, :],
                               in_=kre[q2 * 16:(q2 + 1) * 16, :])
                sync.dma_start(out=krep[q2 * 32 + 16:q2 * 32 + 32, :],
                               in_=kro[q2 * 16:(q2 + 1) * 16, :])
            # v' tiles: [keys, 64+ones]
            ve.memset(vps[:], 1.0)
            for kt in range(16):
                sync.dma_start_transpose(out=vps[:, kt * 80:kt * 80 + 64],
                                         in_=vT[:, kt * 128:(kt + 1) * 128])

            if DEBUG_TAPS:
                sync.dma_start(out=dbg["d_re"].ap(), in_=qre[:])
                sync.dma_start(out=dbg["d_ro"].ap(), in_=qro[:])
                sync.dma_start(out=dbg["d_krep"].ap(), in_=krep[:])
                sync.dma_start(out=dbg["d_qrep0"].ap(), in_=qrep[0][:])
                sync.dma_start(out=dbg["d_vps"].ap(), in_=vps[:])

            # ---------- attention ----------
            for st in range(NST):
                y2b = yw.tile([128, 512], BF16, tag="y2b")
                for h in range(2):
                    u = ps.tile([65, 1024], F32, tag="u")
                    nkt = 4 * st + 4
                    for kt in range(nkt):
                        j = kt - 4 * st          # >=0 on the diagonal group
                        off = max(j, 0) * 128
                        sc = ps2.tile([128, 1024], F32, tag="score")
                        qs = slice(st * 512 + off, (st + 1) * 512)
                        ks = slice(kt * 128, (kt + 1) * 128)
                        te.matmul(out=sc[:, off:512], lhsT=krep[0:32, ks],
                                  rhs=qrep[h][0:32, qs], start=True, stop=True)
                        te.matmul(out=sc[:, 512 + off:1024], lhsT=krep[32:64, ks],
                                  rhs=qrep[h][32:64, qs], start=True, stop=True)
                        e = ework.tile([128, 1024], BF16, tag="e")
                        se.activation(out=e[:], in_=sc[:], func=AF.Exp, scale=SCALE)
                        if j >= 0:
                            if off:
                                ve.memset(e[:, 0:off], 0.0)
                                ve.memset(e[:, 512:512 + off], 0.0)
                            ve.tensor_tensor(out=e[:, off:off + 128],
                                             in0=e[:, off:off + 128], in1=tri[:],
                                             op=OP.mult)
                            ve.tensor_tensor(out=e[:, 512 + off:640 + off],
                                             in0=e[:, 512 + off:640 + off],
                                             in1=tri[:], op=OP.mult)
                        if DEBUG_TAPS and st == 0 and h == 0 and kt == 0:
                            sync.dma_start(out=dbg["d_e"].ap(), in_=e[:])
                        vp = vps[:, kt * 80:kt * 80 + 65]
                        te.matmul(out=u[:, 0:512], lhsT=vp, rhs=e[:, 0:512],
                                  start=(kt == 0), stop=(kt == nkt - 1))
                        te.matmul(out=u[:, 512:1024], lhsT=vp, rhs=e[:, 512:1024],
                                  start=(kt == 0), stop=(kt == nkt - 1))
                    # y~ = Z2*u1 - lam*Z1*u2  (scale-invariant under RMSNorm)
                    zrow = yw.tile([1, 1024], F32, tag="zrow")
                    ve.tensor_copy(out=zrow[:], in_=u[64:65, :])
                    if DEBUG_TAPS and st == 0 and h == 0:
                        sync.dma_start(out=dbg["d_zrow"].ap(), in_=zrow[:])
                    z2b = yw.tile([64, 512], F32, tag="z2b")
                    z1b = yw.tile([64, 512], F32, tag="z1b")
                    gp.partition_broadcast(z2b[:], zrow[0:1, 512:1024])
                    gp.partition_broadcast(z1b[:], zrow[0:1, 0:512])
                    t1 = yw.tile([64, 512], BF16, tag="yt1")
                    t2 = yw.tile([64, 512], BF16, tag="yt2")
                    ve.tensor_tensor(out=t1[:], in0=u[0:64, 0:512], in1=z2b[:],
                                     op=OP.mult)
                    ve.scalar_tensor_tensor(out=t2[:], in0=u[0:64, 512:1024],
                                            scalar=neglam64[h][:], in1=z1b[:],
                                            op0=OP.mult, op1=OP.mult)
                    ys = ybuf[h * 64:(h + 1) * 64, st * 512:(st + 1) * 512]
                    ve.tensor_tensor(out=ys, in0=t1[:], in1=t2[:], op=OP.add)
                    ve.tensor_tensor(out=y2b[h * 64:(h + 1) * 64, :], in0=ys,
                                     in1=ys, op=OP.mult)
                for h in range(2):
                    ssq = ps.tile([1, 512], F32, tag="ssq")
                    te.matmul(out=ssq[:], lhsT=ones2[:, h:h + 1], rhs=y2b[:],
                              start=True, stop=True)
                    ve.tensor_copy(
                        out=ssqbuf[:, (st % 2) * 1024 + h * 512: (st % 2) * 1024 + (h + 1) * 512],
                        in_=ssq[:])

                # ---------- per T-half: rstd, normalize, A2A, out-proj ----------
                if st % 2 == 1:
                    half = st // 2
                    hs = slice(half * 1024, (half + 1) * 1024)
                    lnb = yw1.tile([1, 2048], F32, tag="lnb")
                    se.activation(out=lnb[:], in_=ssqbuf[:, 0:2048],
                                  func=AF.Ln, scale=1.0 / 64.0, bias=epsb[:])
                    rstd2 = yw1.tile([1, 2048], F32, tag="rstd2")
                    se.activation(out=rstd2[:], in_=lnb[:], func=AF.Exp, scale=-0.5)
                    ynorm = yw1.tile([128, 1024], BF16, tag="ynorm")
                    for sh in range(2):
                        for h in range(2):
                            rsl = yw.tile([1, 512], F32, tag="rsl")
                            ve.tensor_copy(
                                out=rsl[:],
                                in_=rstd2[0:1, (sh * 2 + h) * 512:(sh * 2 + h + 1) * 512])
                            rq = yw.tile([64, 512], F32, tag="rq")
                            gp.partition_broadcast(rq[:], rsl[:])
                            colsl = slice(half * 1024 + sh * 512,
                                          half * 1024 + (sh + 1) * 512)
                            osl = slice(sh * 512, (sh + 1) * 512)
                            if h == 0:
                                ve.tensor_tensor(out=ynorm[0:64, osl],
                                                 in0=ybuf[0:64, colsl], in1=rq[:],
                                                 op=OP.mult)
                            else:
                                ycp = yw.tile([64, 512], BF16, tag="ycp")
                                ve.tensor_copy(out=ycp[:], in_=ybuf[64:128, colsl])
                                ve.tensor_tensor(out=ynorm[64:128, osl],
                                                 in0=ycp[:], in1=rq[:], op=OP.mult)
                    if DEBUG_TAPS and half == 0:
                        sync.dma_start(out=dbg["d_lnb"].ap(), in_=lnb[:])
                        sync.dma_start(out=dbg["d_rstd2"].ap(), in_=rstd2[:])
                        sync.dma_start(out=dbg["d_ynorm"].ap(), in_=ynorm[:])
                    a2a_in = dram.tile([8, 128, 128], BF16, tag="a2ai")
                    a2a_out = dram.tile([8, 128, 128], BF16, tag="a2ao")
                    for j in range(8):
                        sync.dma_start(out=a2a_in[j], in_=ynorm[:, j * 128:(j + 1) * 128])
                    gp.collective_compute(
                        "AllToAll", OP.bypass,
                        replica_groups=[list(range(N_CORES))],
                        ins=[a2a_in[:].opt()], outs=[a2a_out[:].opt()])
                    yTloc = yw1.tile([128, 1024], BF16, tag="yTloc")
                    if DEBUG_TAPS and half == 0:
                        sync.dma_start(out=dbg["d_ybuf"].ap(), in_=ybuf[:])
                        sync.dma_start(out=dbg["d_ssq"].ap(), in_=ssqbuf[:])
                    for j in range(8):
                        sync.dma_start(out=yTloc[:, j * 128:(j + 1) * 128], in_=a2a_out[j])
                    if DEBUG_TAPS and half == 0:
                        sync.dma_start(out=dbg["d_yTloc"].ap(), in_=yTloc[:])
                    for oc in range(2):
                        op_ps = ps.tile([128, 512], F32, tag="mm512")
                        for c in range(8):
                            te.matmul(out=op_ps[:],
                                      lhsT=yTloc[:, c * 128:(c + 1) * 128],
                                      rhs=woutT[:, c * C + oc * 512: c * C + (oc + 1) * 512],
                                      start=(c == 0), stop=(c == 7))
                        ob = yw1.tile([128, 512], F32, tag="ob")
                        ve.tensor_copy(out=ob[:], in_=op_ps[:])
                        sync.dma_start(
                            out=out_d.ap()[half * 128:(half + 1) * 128,
                                           oc * 512:(oc + 1) * 512],
                            in_=ob[:])

    nc.compile()
    return nc


_NC_CACHE = None


def _get_nc():
    global _NC_CACHE
    if _NC_CACHE is None:
        _NC_CACHE = _build_nc()
    return _NC_CACHE


def make_in_maps(x, Wq, Wkv, Wout, lambda_init, rms_weight):
    x2 = np.ascontiguousarray(x.reshape(T, C), dtype=np.float32)
    wout = np.ascontiguousarray(Wout, dtype=np.float32)
    inv_freq = 1.0 / (10000.0 ** (np.arange(0, HALF, 2, dtype=np.float32) / HALF))
    t = np.arange(T, dtype=np.float32)
    freqs = np.outer(inv_freq, t)                       # [16, T]
    cos16 = np.ascontiguousarray(np.cos(freqs), dtype=NPBF16)
    sin16 = np.ascontiguousarray(np.sin(freqs), dtype=NPBF16)
    kk, qq = np.meshgrid(np.arange(128), np.arange(128), indexing="ij")
    tri = np.ascontiguousarray((kk <= qq), dtype=NPBF16)
    ones2 = np.zeros((128, 2), dtype=NPBF16)
    ones2[0:64, 0] = 1
    ones2[64:128, 1] = 1
    maps = []
    for c in range(N_CORES):
        g = c // 2
        wkv_rows = np.concatenate(
            [Wkv[g * HD:(g + 1) * HD], Wkv[256 + g * HD:256 + (g + 1) * HD]], 0)
        maps.append({
            "x": x2,
            "wq": np.ascontiguousarray(Wq[c * 128:(c + 1) * 128], np.float32),
            "wkv": np.ascontiguousarray(wkv_rows, np.float32),
            "wout": wout,
            "lam2": np.ascontiguousarray(
                lambda_init[2 * c:2 * c + 2].reshape(1, 2), np.float32),
            "rmsw": np.ascontiguousarray(rms_weight.reshape(64, 1), np.float32),
            "cos16": cos16,
            "sin16": sin16,
            "tri": tri,
            "ones2": ones2,
        })
    return maps


def assemble_out(results):
    out = np.empty((T, C), dtype=np.float32)
    for c in range(N_CORES):
        blk = results[c]["out"]
        out[c * 128:(c + 1) * 128] = blk[0:128]
        out[1024 + c * 128:1024 + (c + 1) * 128] = blk[128:256]
    return out.reshape(1, T, C)


def kernel(x, Wq, Wkv, Wout, lambda_init, rms_weight):
    nc = _get_nc()
    in_maps = make_in_maps(np.asarray(x), np.asarray(Wq), np.asarray(Wkv),
                           np.asarray(Wout), np.asarray(lambda_init),
                           np.asarray(rms_weight))
    res = bass_utils.run_bass_kernel_spmd(nc, in_maps,
                                          core_ids=list(range(N_CORES)))
    return assemble_out(res.results)
